# revision 19
# baseline (speedup 1.0000x reference)
"""Trainium2 Bass kernel for nn_DenseBlockEnd (gnn_message_passing).

Computes, for each graph b (B=512, MAX_ATOM=256, F=256):
    out[b] = relu(mask[b] * (node[b] + sum_l beta1*A_l[b] @ W_in[l]
                                     + beta2*BO[b] @ W_out[0]))
with mask[b, m] = (m < mol_slice[b]).

Strategy (memory-roofline): rows with m >= mol_slice[b] are exactly zero in
the output and never read, so the host packs only the VALID rows (about half
of them on average), balanced across the 8 cores, and scatters the device
results back into a zero-filled full output.  All device-side tensors are
pre-cast to bf16 and pre-transposed on the host into a uniform
[o_half, 128, rows] feature-on-partition layout, so the device does no
transposes at all: W chunks are the stationary matmul operand, packed
activation rows stream through the PE, node rows are added on the Vector
engine and relu+bf16-store happens on the Scalar engine.  Device HBM traffic
drops from ~80 MB/core (dense f32) to ~22 MB/core (valid rows, bf16).
"""

import numpy as np
import ml_dtypes
from contextlib import ExitStack

import concourse.bass as bass
import concourse.tile as tile
from concourse import bacc, mybir
from concourse import bass_utils

B, M, F = 512, 256, 256
NCORES = 8
NSLAB = 3                 # inblock_acts[0], inblock_acts[1], block_outputs[0]
P = 128
RC = 1024                 # rows per pipeline chunk
ROW_PAD = 256             # per-core row count rounded up to this

F32 = mybir.dt.float32
BF16 = mybir.dt.bfloat16
BF16_NP = ml_dtypes.bfloat16

_nc_cache = {}


def _build_nc(ntot):
    nc = bacc.Bacc(trn_type="TRN2", target_bir_lowering=False, debug=False)

    # acts: 6 combos c = slab*2 + f_chunk, each [128 f, ntot rows]
    a_d = nc.dram_tensor("acts", [2 * NSLAB, P, ntot], BF16, kind="ExternalInput").ap()
    node_d = nc.dram_tensor("nodet", [2, P, ntot], BF16, kind="ExternalInput").ap()
    wc_d = nc.dram_tensor("wc", [2 * NSLAB, P, F], BF16, kind="ExternalInput").ap()
    ident_d = nc.dram_tensor("ident", [P, P], BF16, kind="ExternalInput").ap()
    out_d = nc.dram_tensor("out", [2, P, ntot], BF16, kind="ExternalOutput").ap()

    # Uniform big chunks: DMA efficiency dominates; compute-start latency is
    # handled by splitting each chunk's activation load in halves.
    sizes = [RC] * (ntot // RC) + ([ntot % RC] if ntot % RC else [])
    chunks = []
    r0 = 0
    for rc in sizes:
        chunks.append((r0, rc))
        r0 += rc
    assert r0 == ntot

    with tile.TileContext(nc) as tc, ExitStack() as ctx:
        const_pool = ctx.enter_context(tc.tile_pool(name="const", bufs=1))
        at_pool = ctx.enter_context(tc.tile_pool(name="at", bufs=5))
        nd_pool = ctx.enter_context(tc.tile_pool(name="nd", bufs=5))
        out_pool = ctx.enter_context(tc.tile_pool(name="outp", bufs=4))
        psum_pool = ctx.enter_context(tc.tile_pool(name="psum", bufs=6, space="PSUM"))

        # Stationary weights: w_sb[p_f, c, o] = (beta * W)[c//2][(c%2)*128 + p_f, o]
        w_sb = const_pool.tile([P, 2 * NSLAB, F], BF16, name="w_sb")
        nc.sync.dma_start(w_sb[:], wc_d.rearrange("c p o -> p c o"))
        ident_sb = const_pool.tile([P, P], BF16, name="ident_sb")
        nc.sync.dma_start(ident_sb[:], ident_d[:])
        # Scratch PSUM for keep-warm dummy matmuls (never read).
        warm_pool = ctx.enter_context(tc.tile_pool(name="warm", bufs=1, space="PSUM"))
        warm_ps = warm_pool.tile([P, 256], F32, name="warm_ps")

        tiles = {}

        def load_chunk(ci, queue, split):
            r0, rc = chunks[ci]
            at = at_pool.tile([P, 2 * NSLAB, RC], BF16, name="at", tag="at")
            nd = nd_pool.tile([P, 2, RC], BF16, name="nd", tag="nd")
            step = (2 * NSLAB) // split
            for h in range(split):
                c0 = h * step
                queue.dma_start(
                    at[:, c0 : c0 + step, :rc],
                    a_d[c0 : c0 + step, :, r0 : r0 + rc].rearrange("c p r -> p c r"),
                )
            queue.dma_start(
                nd[:, :, :rc], node_d[:, :, r0 : r0 + rc].rearrange("c p r -> p c r")
            )
            tiles[ci] = (at, nd)

        def compute_chunk(ci, rblock, wq):
            r0, rc = chunks[ci]
            at, nd = tiles[ci]
            ot = out_pool.tile([P, 2, RC], BF16, name="ot", tag="ot")
            parity = 0
            for j in range(2):          # output-feature half (psum partition dim)
                nrb = (rc + rblock - 1) // rblock
                for rb in range(nrb):   # row blocks (<=512 rows: one PSUM bank)
                    o0 = rb * rblock
                    n = min(rblock, rc - o0)
                    ps = psum_pool.tile([P, 512], F32, name="ps", tag="ps")
                    for c in range(2 * NSLAB):
                        nc.tensor.matmul(
                            ps[:, :n],
                            w_sb[:, c, j * P : (j + 1) * P],
                            at[:, c, o0 : o0 + n],
                            start=(c == 0),
                            stop=False,
                        )
                    # node-add folded into the accumulation: += I.T @ node_half
                    nc.tensor.matmul(
                        ps[:, :n],
                        ident_sb[:],
                        nd[:, j, o0 : o0 + n],
                        start=False,
                        stop=True,
                    )
                    # relu + bf16 store, alternating engines so adjacent
                    # blocks drain PSUM concurrently
                    if parity == 0:
                        nc.scalar.activation(
                            ot[:, j, o0 : o0 + n],
                            ps[:, :n],
                            mybir.ActivationFunctionType.Relu,
                        )
                    else:
                        nc.vector.tensor_scalar_max(
                            ot[:, j, o0 : o0 + n], ps[:, :n], 0.0
                        )
                    parity ^= 1
                # Store per feature-half so the write drains while the other
                # half computes.
                wq.dma_start(
                    out_d[j : j + 1, :, r0 : r0 + rc].rearrange("c p r -> p c r"),
                    ot[:, j : j + 1, :rc],
                )
            # keep-warm: stop HAM from re-throttling the PE during the
            # chunk-boundary DMA wait (result never read)
            if ci < len(chunks) - 1:
                nc.tensor.matmul(
                    warm_ps[:, :256], w_sb[:, 0, 0:P], w_sb[:, 0, 0:256],
                    start=True, stop=True,
                )

        nchunk = len(chunks)
        for ci in range(nchunk):
            load_chunk(ci, nc.sync, split=2)
        for ci in range(nchunk):
            last = ci == nchunk - 1
            compute_chunk(ci, rblock=512,
                          wq=nc.scalar if last else nc.gpsimd)

    nc.compile()
    return nc


def get_nc(ntot):
    if ntot not in _nc_cache:
        _nc_cache[ntot] = _build_nc(ntot)
    return _nc_cache[ntot]


def _plan(mol):
    """Balance graphs across cores by valid-row count; build gather indices."""
    mol = np.asarray(mol, dtype=np.int64)
    order = np.argsort(-mol, kind="stable")
    loads = np.zeros(NCORES, dtype=np.int64)
    groups = [[] for _ in range(NCORES)]
    for b in order:
        c = int(np.argmin(loads))
        groups[c].append(int(b))
        loads[c] += mol[b]
    ntot = int(-(-loads.max() // ROW_PAD) * ROW_PAD)
    idx = np.zeros((NCORES, ntot), dtype=np.int64)
    nvalid = np.zeros(NCORES, dtype=np.int64)
    for c in range(NCORES):
        ids = np.concatenate(
            [b * M + np.arange(mol[b]) for b in groups[c]]
        ) if groups[c] else np.zeros(0, dtype=np.int64)
        idx[c, : len(ids)] = ids
        nvalid[c] = len(ids)
    return {"ntot": ntot, "idx": idx, "nvalid": nvalid}


def _packT(flat2d, idx):
    """Gather rows [8, ntot, 256] then lay out as [8, 2, 128, ntot] bf16."""
    g = flat2d[idx]                      # [8, ntot, 256]
    gt = g.transpose(0, 2, 1)            # [8, 256, ntot] (view)
    return np.ascontiguousarray(gt.astype(BF16_NP)).reshape(
        NCORES, 2, P, idx.shape[1]
    )


def plan_and_pack(
    node_features,
    inblock_acts,
    block_outputs,
    mol_slice,
    W_in,
    W_out,
    beta1,
    beta2,
):
    node = np.asarray(node_features, dtype=np.float32).reshape(B * M, F)
    inb = np.asarray(inblock_acts, dtype=np.float32)
    bo = np.asarray(block_outputs, dtype=np.float32)
    mol = np.asarray(mol_slice, dtype=np.int64)
    w_in = np.asarray(W_in, dtype=np.float32)
    w_out = np.asarray(W_out, dtype=np.float32)
    b1 = float(np.asarray(beta1).reshape(-1)[0])
    b2 = float(np.asarray(beta2).reshape(-1)[0])

    plan = _plan(mol)
    idx = plan["idx"]

    wc = (
        np.concatenate([b1 * w_in[0], b1 * w_in[1], b2 * w_out[0]], axis=0)
        .reshape(2 * NSLAB, P, F)
        .astype(BF16_NP)
    )

    nodeT = _packT(node, idx)
    a0T = _packT(inb[0].reshape(B * M, F), idx)
    a1T = _packT(inb[1].reshape(B * M, F), idx)
    boT = _packT(bo[0].reshape(B * M, F), idx)
    ntot = plan["ntot"]
    ident = np.eye(P, dtype=BF16_NP)

    in_maps = []
    for c in range(NCORES):
        acts = np.empty((2 * NSLAB, P, ntot), dtype=BF16_NP)
        acts[0:2] = a0T[c]
        acts[2:4] = a1T[c]
        acts[4:6] = boT[c]
        in_maps.append(
            {
                "acts": acts,
                "nodet": np.ascontiguousarray(nodeT[c]),
                "wc": wc,
                "ident": ident,
            }
        )
    return plan, in_maps


def unpack(plan, per_core_outs):
    idx, nvalid = plan["idx"], plan["nvalid"]
    ntot = plan["ntot"]
    out_flat = np.zeros((B * M, F), dtype=np.float32)
    for c in range(NCORES):
        o = np.asarray(per_core_outs[c]).reshape(F, ntot)  # [2,128,ntot]->[256,ntot]
        nv = int(nvalid[c])
        out_flat[idx[c, :nv]] = o[:, :nv].T.astype(np.float32)
    return out_flat.reshape(B, M, F)


def kernel(**inputs):
    plan, in_maps = plan_and_pack(**inputs)
    nc = get_nc(plan["ntot"])
    res = bass_utils.run_bass_kernel_spmd(
        nc, in_maps, core_ids=list(range(NCORES))
    )
    return unpack(plan, [res.results[c]["out"] for c in range(NCORES)])


# revision 23
# speedup vs baseline: 1.0128x; 1.0128x over previous
"""Trainium2 Bass kernel for nn_DenseBlockEnd (gnn_message_passing).

Computes, for each graph b (B=512, MAX_ATOM=256, F=256):
    out[b] = relu(mask[b] * (node[b] + sum_l beta1*A_l[b] @ W_in[l]
                                     + beta2*BO[b] @ W_out[0]))
with mask[b, m] = (m < mol_slice[b]).

Strategy (memory-roofline): rows with m >= mol_slice[b] are exactly zero in
the output and never read, so the host packs only the VALID rows (about half
of them on average), balanced across the 8 cores, and scatters the device
results back into a zero-filled full output.  All device-side tensors are
pre-cast to bf16 and pre-transposed on the host into a uniform
[o_half, 128, rows] feature-on-partition layout, so the device does no
transposes at all: W chunks are the stationary matmul operand, packed
activation rows stream through the PE, node rows are added on the Vector
engine and relu+bf16-store happens on the Scalar engine.  Device HBM traffic
drops from ~80 MB/core (dense f32) to ~22 MB/core (valid rows, bf16).
"""

import numpy as np
import ml_dtypes
from contextlib import ExitStack

import concourse.bass as bass
import concourse.tile as tile
from concourse import bacc, mybir
from concourse import bass_utils

B, M, F = 512, 256, 256
NCORES = 8
NSLAB = 3                 # inblock_acts[0], inblock_acts[1], block_outputs[0]
P = 128
RC = 1024                 # rows per pipeline chunk
ROW_PAD = 256             # per-core row count rounded up to this

F32 = mybir.dt.float32
BF16 = mybir.dt.bfloat16
BF16_NP = ml_dtypes.bfloat16

_nc_cache = {}


def _build_nc(ntot):
    nc = bacc.Bacc(trn_type="TRN2", target_bir_lowering=False, debug=False)

    # acts: 6 combos c = slab*2 + f_chunk, each [128 f, ntot rows]
    a_d = nc.dram_tensor("acts", [2 * NSLAB, P, ntot], BF16, kind="ExternalInput").ap()
    node_d = nc.dram_tensor("nodet", [2, P, ntot], BF16, kind="ExternalInput").ap()
    wc_d = nc.dram_tensor("wc", [2 * NSLAB, P, F], BF16, kind="ExternalInput").ap()
    ident_d = nc.dram_tensor("ident", [P, P], BF16, kind="ExternalInput").ap()
    out_d = nc.dram_tensor("out", [2, P, ntot], BF16, kind="ExternalOutput").ap()

    # Uniform big chunks: DMA efficiency dominates; compute-start latency is
    # handled by splitting each chunk's activation load in halves.
    sizes = [RC] * (ntot // RC) + ([ntot % RC] if ntot % RC else [])
    chunks = []
    r0 = 0
    for rc in sizes:
        chunks.append((r0, rc))
        r0 += rc
    assert r0 == ntot

    with tile.TileContext(nc) as tc, ExitStack() as ctx:
        const_pool = ctx.enter_context(tc.tile_pool(name="const", bufs=1))
        at_pool = ctx.enter_context(tc.tile_pool(name="at", bufs=5))
        nd_pool = ctx.enter_context(tc.tile_pool(name="nd", bufs=5))
        out_pool = ctx.enter_context(tc.tile_pool(name="outp", bufs=4))
        psum_pool = ctx.enter_context(tc.tile_pool(name="psum", bufs=6, space="PSUM"))

        # Stationary weights: w_sb[p_f, c, o] = (beta * W)[c//2][(c%2)*128 + p_f, o]
        w_sb = const_pool.tile([P, 2 * NSLAB, F], BF16, name="w_sb")
        nc.sync.dma_start(w_sb[:], wc_d.rearrange("c p o -> p c o"))
        ident_sb = const_pool.tile([P, P], BF16, name="ident_sb")
        nc.sync.dma_start(ident_sb[:], ident_d[:])
        # Scratch PSUM for keep-warm dummy matmuls (never read).
        warm_pool = ctx.enter_context(tc.tile_pool(name="warm", bufs=1, space="PSUM"))
        warm_ps = warm_pool.tile([P, 256], F32, name="warm_ps")

        tiles = {}

        def load_chunk(ci, queue, split):
            r0, rc = chunks[ci]
            at = at_pool.tile([P, 2 * NSLAB, RC], BF16, name="at", tag="at")
            nd = nd_pool.tile([P, 2, RC], BF16, name="nd", tag="nd")
            step = (2 * NSLAB) // split
            for h in range(split):
                c0 = h * step
                queue.dma_start(
                    at[:, c0 : c0 + step, :rc],
                    a_d[c0 : c0 + step, :, r0 : r0 + rc].rearrange("c p r -> p c r"),
                )
            queue.dma_start(
                nd[:, :, :rc], node_d[:, :, r0 : r0 + rc].rearrange("c p r -> p c r")
            )
            tiles[ci] = (at, nd)

        def compute_chunk(ci, rblock, wq):
            r0, rc = chunks[ci]
            at, nd = tiles[ci]
            ot = out_pool.tile([P, 2, RC], BF16, name="ot", tag="ot")
            parity = 0
            for j in range(2):          # output-feature half (psum partition dim)
                nrb = (rc + rblock - 1) // rblock
                for rb in range(nrb):   # row blocks (<=512 rows: one PSUM bank)
                    o0 = rb * rblock
                    n = min(rblock, rc - o0)
                    ps = psum_pool.tile([P, 512], F32, name="ps", tag="ps")
                    for c in range(2 * NSLAB):
                        nc.tensor.matmul(
                            ps[:, :n],
                            w_sb[:, c, j * P : (j + 1) * P],
                            at[:, c, o0 : o0 + n],
                            start=(c == 0),
                            stop=(c == 2 * NSLAB - 1),
                        )
                    nc.vector.tensor_add(
                        ps[:, :n], ps[:, :n], nd[:, j, o0 : o0 + n]
                    )
                    nc.scalar.activation(
                        ot[:, j, o0 : o0 + n],
                        ps[:, :n],
                        mybir.ActivationFunctionType.Relu,
                    )
                # Store per feature-half so the write drains while the other
                # half computes.
                wq.dma_start(
                    out_d[j : j + 1, :, r0 : r0 + rc].rearrange("c p r -> p c r"),
                    ot[:, j : j + 1, :rc],
                )
            # keep-warm: stop HAM from re-throttling the PE during the
            # chunk-boundary DMA wait (result never read)
            if ci < len(chunks) - 1:
                nc.tensor.matmul(
                    warm_ps[:, :256], w_sb[:, 0, 0:P], w_sb[:, 0, 0:256],
                    start=True, stop=True,
                )

        nchunk = len(chunks)
        for ci in range(nchunk):
            load_chunk(ci, nc.sync, split=2 if ci == 0 else 1)
        for ci in range(nchunk):
            last = ci == nchunk - 1
            compute_chunk(ci, rblock=512,
                          wq=nc.scalar if last else nc.gpsimd)

    nc.compile()
    return nc


def get_nc(ntot):
    if ntot not in _nc_cache:
        _nc_cache[ntot] = _build_nc(ntot)
    return _nc_cache[ntot]


def _plan(mol):
    """Balance graphs across cores by valid-row count; build gather indices."""
    mol = np.asarray(mol, dtype=np.int64)
    order = np.argsort(-mol, kind="stable")
    loads = np.zeros(NCORES, dtype=np.int64)
    groups = [[] for _ in range(NCORES)]
    for b in order:
        c = int(np.argmin(loads))
        groups[c].append(int(b))
        loads[c] += mol[b]
    ntot = int(-(-loads.max() // ROW_PAD) * ROW_PAD)
    idx = np.zeros((NCORES, ntot), dtype=np.int64)
    nvalid = np.zeros(NCORES, dtype=np.int64)
    for c in range(NCORES):
        ids = np.concatenate(
            [b * M + np.arange(mol[b]) for b in groups[c]]
        ) if groups[c] else np.zeros(0, dtype=np.int64)
        idx[c, : len(ids)] = ids
        nvalid[c] = len(ids)
    return {"ntot": ntot, "idx": idx, "nvalid": nvalid}


def _packT(flat2d, idx):
    """Gather rows [8, ntot, 256] then lay out as [8, 2, 128, ntot] bf16."""
    g = flat2d[idx]                      # [8, ntot, 256]
    gt = g.transpose(0, 2, 1)            # [8, 256, ntot] (view)
    return np.ascontiguousarray(gt.astype(BF16_NP)).reshape(
        NCORES, 2, P, idx.shape[1]
    )


def plan_and_pack(
    node_features,
    inblock_acts,
    block_outputs,
    mol_slice,
    W_in,
    W_out,
    beta1,
    beta2,
):
    node = np.asarray(node_features, dtype=np.float32).reshape(B * M, F)
    inb = np.asarray(inblock_acts, dtype=np.float32)
    bo = np.asarray(block_outputs, dtype=np.float32)
    mol = np.asarray(mol_slice, dtype=np.int64)
    w_in = np.asarray(W_in, dtype=np.float32)
    w_out = np.asarray(W_out, dtype=np.float32)
    b1 = float(np.asarray(beta1).reshape(-1)[0])
    b2 = float(np.asarray(beta2).reshape(-1)[0])

    plan = _plan(mol)
    idx = plan["idx"]

    wc = (
        np.concatenate([b1 * w_in[0], b1 * w_in[1], b2 * w_out[0]], axis=0)
        .reshape(2 * NSLAB, P, F)
        .astype(BF16_NP)
    )

    nodeT = _packT(node, idx)
    a0T = _packT(inb[0].reshape(B * M, F), idx)
    a1T = _packT(inb[1].reshape(B * M, F), idx)
    boT = _packT(bo[0].reshape(B * M, F), idx)
    ntot = plan["ntot"]
    ident = np.eye(P, dtype=BF16_NP)

    in_maps = []
    for c in range(NCORES):
        acts = np.empty((2 * NSLAB, P, ntot), dtype=BF16_NP)
        acts[0:2] = a0T[c]
        acts[2:4] = a1T[c]
        acts[4:6] = boT[c]
        in_maps.append(
            {
                "acts": acts,
                "nodet": np.ascontiguousarray(nodeT[c]),
                "wc": wc,
                "ident": ident,
            }
        )
    return plan, in_maps


def unpack(plan, per_core_outs):
    idx, nvalid = plan["idx"], plan["nvalid"]
    ntot = plan["ntot"]
    out_flat = np.zeros((B * M, F), dtype=np.float32)
    for c in range(NCORES):
        o = np.asarray(per_core_outs[c]).reshape(F, ntot)  # [2,128,ntot]->[256,ntot]
        nv = int(nvalid[c])
        out_flat[idx[c, :nv]] = o[:, :nv].T.astype(np.float32)
    return out_flat.reshape(B, M, F)


def kernel(**inputs):
    plan, in_maps = plan_and_pack(**inputs)
    nc = get_nc(plan["ntot"])
    res = bass_utils.run_bass_kernel_spmd(
        nc, in_maps, core_ids=list(range(NCORES))
    )
    return unpack(plan, [res.results[c]["out"] for c in range(NCORES)])


# revision 28
# speedup vs baseline: 1.0437x; 1.0305x over previous
"""Trainium2 Bass kernel for nn_DenseBlockEnd (gnn_message_passing).

Computes, for each graph b (B=512, MAX_ATOM=256, F=256):
    out[b] = relu(mask[b] * (node[b] + sum_l beta1*A_l[b] @ W_in[l]
                                     + beta2*BO[b] @ W_out[0]))
with mask[b, m] = (m < mol_slice[b]).

Strategy (memory-roofline): rows with m >= mol_slice[b] are exactly zero in
the output and never read, so the host packs only the VALID rows (about half
of them on average), balanced across the 8 cores, and scatters the device
results back into a zero-filled full output.  All device-side tensors are
pre-cast to bf16 and pre-transposed on the host into a uniform
[o_half, 128, rows] feature-on-partition layout, so the device does no
transposes at all: W chunks are the stationary matmul operand, packed
activation rows stream through the PE, node rows are added on the Vector
engine and relu+bf16-store happens on the Scalar engine.  Device HBM traffic
drops from ~80 MB/core (dense f32) to ~22 MB/core (valid rows, bf16).
"""

import numpy as np
import ml_dtypes
from contextlib import ExitStack

import concourse.bass as bass
import concourse.tile as tile
from concourse import bacc, mybir
from concourse import bass_utils

B, M, F = 512, 256, 256
NCORES = 8
NSLAB = 3                 # inblock_acts[0], inblock_acts[1], block_outputs[0]
P = 128
RC = 1024                 # rows per pipeline chunk
ROW_PAD = 256             # per-core row count rounded up to this

F32 = mybir.dt.float32
BF16 = mybir.dt.bfloat16
BF16_NP = ml_dtypes.bfloat16

_nc_cache = {}


def _build_nc(ntot):
    nc = bacc.Bacc(trn_type="TRN2", target_bir_lowering=False, debug=False)

    # acts: 6 combos c = slab*2 + f_chunk, each [128 f, ntot rows]
    a_d = nc.dram_tensor("acts", [2 * NSLAB, P, ntot], BF16, kind="ExternalInput").ap()
    node_d = nc.dram_tensor("nodet", [2, P, ntot], BF16, kind="ExternalInput").ap()
    wc_d = nc.dram_tensor("wc", [2 * NSLAB, P, F], BF16, kind="ExternalInput").ap()
    out_d = nc.dram_tensor("out", [2, P, ntot], BF16, kind="ExternalOutput").ap()

    # Uniform big chunks: DMA efficiency dominates; compute-start latency is
    # handled by splitting each chunk's activation load in halves.
    sizes = [RC] * (ntot // RC) + ([ntot % RC] if ntot % RC else [])
    chunks = []
    r0 = 0
    for rc in sizes:
        chunks.append((r0, rc))
        r0 += rc
    assert r0 == ntot

    with tile.TileContext(nc) as tc, ExitStack() as ctx:
        const_pool = ctx.enter_context(tc.tile_pool(name="const", bufs=1))
        at_pool = ctx.enter_context(tc.tile_pool(name="at", bufs=5))
        nd_pool = ctx.enter_context(tc.tile_pool(name="nd", bufs=5))
        out_pool = ctx.enter_context(tc.tile_pool(name="outp", bufs=4))
        psum_pool = ctx.enter_context(tc.tile_pool(name="psum", bufs=6, space="PSUM"))

        # Stationary weights: w_sb[p_f, c, o] = (beta * W)[c//2][(c%2)*128 + p_f, o]
        w_sb = const_pool.tile([P, 2 * NSLAB, F], BF16, name="w_sb")
        nc.sync.dma_start(w_sb[:], wc_d.rearrange("c p o -> p c o"))

        tiles = {}

        def load_chunk(ci, queue, split):
            r0, rc = chunks[ci]
            at = at_pool.tile([P, 2 * NSLAB, RC], BF16, name="at", tag="at")
            nd = nd_pool.tile([P, 2, RC], BF16, name="nd", tag="nd")
            step = (2 * NSLAB) // split
            for h in range(split):
                c0 = h * step
                queue.dma_start(
                    at[:, c0 : c0 + step, :rc],
                    a_d[c0 : c0 + step, :, r0 : r0 + rc].rearrange("c p r -> p c r"),
                )
            queue.dma_start(
                nd[:, :, :rc], node_d[:, :, r0 : r0 + rc].rearrange("c p r -> p c r")
            )
            tiles[ci] = (at, nd)

        def compute_chunk(ci, rblock, wq):
            r0, rc = chunks[ci]
            at, nd = tiles[ci]
            ot = out_pool.tile([P, 2, RC], BF16, name="ot", tag="ot")
            for j in range(2):          # output-feature half (psum partition dim)
                nrb = (rc + rblock - 1) // rblock
                for rb in range(nrb):   # row blocks (<=512 rows: one PSUM bank)
                    o0 = rb * rblock
                    n = min(rblock, rc - o0)
                    ps = psum_pool.tile([P, 512], F32, name="ps", tag="ps")
                    for c in range(2 * NSLAB):
                        nc.tensor.matmul(
                            ps[:, :n],
                            w_sb[:, c, j * P : (j + 1) * P],
                            at[:, c, o0 : o0 + n],
                            start=(c == 0),
                            stop=(c == 2 * NSLAB - 1),
                        )
                    nc.vector.tensor_add(
                        ps[:, :n], ps[:, :n], nd[:, j, o0 : o0 + n]
                    )
                    nc.scalar.activation(
                        ot[:, j, o0 : o0 + n],
                        ps[:, :n],
                        mybir.ActivationFunctionType.Relu,
                    )
                # Store per feature-half so the write drains while the other
                # half computes.
                wq.dma_start(
                    out_d[j : j + 1, :, r0 : r0 + rc].rearrange("c p r -> p c r"),
                    ot[:, j : j + 1, :rc],
                )

        nchunk = len(chunks)
        for ci in range(nchunk):
            load_chunk(ci, nc.sync, split=2 if ci == 0 else 1)
        for ci in range(nchunk):
            compute_chunk(ci, rblock=512, wq=nc.gpsimd)

    nc.compile()
    return nc


def get_nc(ntot):
    if ntot not in _nc_cache:
        _nc_cache[ntot] = _build_nc(ntot)
    return _nc_cache[ntot]


def _plan(mol):
    """Balance graphs across cores by valid-row count; build gather indices."""
    mol = np.asarray(mol, dtype=np.int64)
    order = np.argsort(-mol, kind="stable")
    loads = np.zeros(NCORES, dtype=np.int64)
    groups = [[] for _ in range(NCORES)]
    for b in order:
        c = int(np.argmin(loads))
        groups[c].append(int(b))
        loads[c] += mol[b]
    ntot = int(-(-loads.max() // ROW_PAD) * ROW_PAD)
    idx = np.zeros((NCORES, ntot), dtype=np.int64)
    nvalid = np.zeros(NCORES, dtype=np.int64)
    for c in range(NCORES):
        ids = np.concatenate(
            [b * M + np.arange(mol[b]) for b in groups[c]]
        ) if groups[c] else np.zeros(0, dtype=np.int64)
        idx[c, : len(ids)] = ids
        nvalid[c] = len(ids)
    return {"ntot": ntot, "idx": idx, "nvalid": nvalid}


def _packT(flat2d, idx):
    """Gather rows [8, ntot, 256] then lay out as [8, 2, 128, ntot] bf16."""
    g = flat2d[idx]                      # [8, ntot, 256]
    gt = g.transpose(0, 2, 1)            # [8, 256, ntot] (view)
    return np.ascontiguousarray(gt.astype(BF16_NP)).reshape(
        NCORES, 2, P, idx.shape[1]
    )


def plan_and_pack(
    node_features,
    inblock_acts,
    block_outputs,
    mol_slice,
    W_in,
    W_out,
    beta1,
    beta2,
):
    node = np.asarray(node_features, dtype=np.float32).reshape(B * M, F)
    inb = np.asarray(inblock_acts, dtype=np.float32)
    bo = np.asarray(block_outputs, dtype=np.float32)
    mol = np.asarray(mol_slice, dtype=np.int64)
    w_in = np.asarray(W_in, dtype=np.float32)
    w_out = np.asarray(W_out, dtype=np.float32)
    b1 = float(np.asarray(beta1).reshape(-1)[0])
    b2 = float(np.asarray(beta2).reshape(-1)[0])

    plan = _plan(mol)
    idx = plan["idx"]

    wc = (
        np.concatenate([b1 * w_in[0], b1 * w_in[1], b2 * w_out[0]], axis=0)
        .reshape(2 * NSLAB, P, F)
        .astype(BF16_NP)
    )

    nodeT = _packT(node, idx)
    a0T = _packT(inb[0].reshape(B * M, F), idx)
    a1T = _packT(inb[1].reshape(B * M, F), idx)
    boT = _packT(bo[0].reshape(B * M, F), idx)
    ntot = plan["ntot"]

    in_maps = []
    for c in range(NCORES):
        acts = np.empty((2 * NSLAB, P, ntot), dtype=BF16_NP)
        acts[0:2] = a0T[c]
        acts[2:4] = a1T[c]
        acts[4:6] = boT[c]
        in_maps.append(
            {
                "acts": acts,
                "nodet": np.ascontiguousarray(nodeT[c]),
                "wc": wc,
            }
        )
    return plan, in_maps


def unpack(plan, per_core_outs):
    idx, nvalid = plan["idx"], plan["nvalid"]
    ntot = plan["ntot"]
    out_flat = np.zeros((B * M, F), dtype=np.float32)
    for c in range(NCORES):
        o = np.asarray(per_core_outs[c]).reshape(F, ntot)  # [2,128,ntot]->[256,ntot]
        nv = int(nvalid[c])
        out_flat[idx[c, :nv]] = o[:, :nv].T.astype(np.float32)
    return out_flat.reshape(B, M, F)


def kernel(**inputs):
    plan, in_maps = plan_and_pack(**inputs)
    nc = get_nc(plan["ntot"])
    res = bass_utils.run_bass_kernel_spmd(
        nc, in_maps, core_ids=list(range(NCORES))
    )
    return unpack(plan, [res.results[c]["out"] for c in range(NCORES)])


# revision 29
# speedup vs baseline: 1.1282x; 1.0810x over previous
"""Trainium2 Bass kernel for nn_DenseBlockEnd (gnn_message_passing).

Computes, for each graph b (B=512, MAX_ATOM=256, F=256):
    out[b] = relu(mask[b] * (node[b] + sum_l beta1*A_l[b] @ W_in[l]
                                     + beta2*BO[b] @ W_out[0]))
with mask[b, m] = (m < mol_slice[b]).

Strategy (memory-roofline): rows with m >= mol_slice[b] are exactly zero in
the output and never read, so the host packs only the VALID rows (about half
of them on average), balanced across the 8 cores, and scatters the device
results back into a zero-filled full output.  All device-side tensors are
pre-cast to bf16 and pre-transposed on the host into a uniform
[o_half, 128, rows] feature-on-partition layout, so the device does no
transposes at all: W chunks are the stationary matmul operand, packed
activation rows stream through the PE, node rows are added on the Vector
engine and relu+bf16-store happens on the Scalar engine.  Device HBM traffic
drops from ~80 MB/core (dense f32) to ~22 MB/core (valid rows, bf16).
"""

import numpy as np
import ml_dtypes
from contextlib import ExitStack

import concourse.bass as bass
import concourse.tile as tile
from concourse import bacc, mybir
from concourse import bass_utils

B, M, F = 512, 256, 256
NCORES = 8
NSLAB = 3                 # inblock_acts[0], inblock_acts[1], block_outputs[0]
P = 128
RC = 1024                 # rows per pipeline chunk
ROW_PAD = 256             # per-core row count rounded up to this

F32 = mybir.dt.float32
BF16 = mybir.dt.bfloat16
BF16_NP = ml_dtypes.bfloat16

_nc_cache = {}


def _build_nc(ntot):
    nc = bacc.Bacc(trn_type="TRN2", target_bir_lowering=False, debug=False)

    # acts: 6 combos c = slab*2 + f_chunk, each [128 f, ntot rows]
    a_d = nc.dram_tensor("acts", [2 * NSLAB, P, ntot], BF16, kind="ExternalInput").ap()
    node_d = nc.dram_tensor("nodet", [2, P, ntot], BF16, kind="ExternalInput").ap()
    wc_d = nc.dram_tensor("wc", [2 * NSLAB, P, F], BF16, kind="ExternalInput").ap()
    out_d = nc.dram_tensor("out", [2, P, ntot], BF16, kind="ExternalOutput").ap()

    # Uniform big chunks: DMA efficiency dominates; compute-start latency is
    # handled by splitting each chunk's activation load in halves.
    sizes = [RC] * (ntot // RC) + ([ntot % RC] if ntot % RC else [])
    chunks = []
    r0 = 0
    for rc in sizes:
        chunks.append((r0, rc))
        r0 += rc
    assert r0 == ntot

    with tile.TileContext(nc) as tc, ExitStack() as ctx:
        const_pool = ctx.enter_context(tc.tile_pool(name="const", bufs=1))
        at_pool = ctx.enter_context(tc.tile_pool(name="at", bufs=5))
        nd_pool = ctx.enter_context(tc.tile_pool(name="nd", bufs=5))
        out_pool = ctx.enter_context(tc.tile_pool(name="outp", bufs=4))
        psum_pool = ctx.enter_context(tc.tile_pool(name="psum", bufs=6, space="PSUM"))

        # Stationary weights: w_sb[p_f, c, o] = (beta * W)[c//2][(c%2)*128 + p_f, o]
        w_sb = const_pool.tile([P, 2 * NSLAB, F], BF16, name="w_sb")
        nc.sync.dma_start(w_sb[:], wc_d.rearrange("c p o -> p c o"))

        # Pre-warm the PE during the otherwise-dead window before the first
        # chunk lands: ~12 dummy matmuls on scratch push the HAM activity
        # window so the real chunk-0 matmuls run at 2.4 GHz, not 1.2 GHz.
        junk = const_pool.tile([P, 512], BF16, name="junk")
        nc.vector.memset(junk[:], 0.0)
        warm_pool = ctx.enter_context(tc.tile_pool(name="warm", bufs=1, space="PSUM"))
        warm_ps = warm_pool.tile([P, 512], F32, name="warm_ps")
        for _ in range(12):
            nc.tensor.matmul(
                warm_ps[:], junk[:, 0:P], junk[:], start=True, stop=True
            )

        tiles = {}

        def load_chunk(ci, queue, split):
            r0, rc = chunks[ci]
            at = at_pool.tile([P, 2 * NSLAB, RC], BF16, name="at", tag="at")
            nd = nd_pool.tile([P, 2, RC], BF16, name="nd", tag="nd")
            step = (2 * NSLAB) // split
            for h in range(split):
                c0 = h * step
                queue.dma_start(
                    at[:, c0 : c0 + step, :rc],
                    a_d[c0 : c0 + step, :, r0 : r0 + rc].rearrange("c p r -> p c r"),
                )
            queue.dma_start(
                nd[:, :, :rc], node_d[:, :, r0 : r0 + rc].rearrange("c p r -> p c r")
            )
            tiles[ci] = (at, nd)

        def compute_chunk(ci, rblock, wq):
            r0, rc = chunks[ci]
            at, nd = tiles[ci]
            ot = out_pool.tile([P, 2, RC], BF16, name="ot", tag="ot")
            for j in range(2):          # output-feature half (psum partition dim)
                nrb = (rc + rblock - 1) // rblock
                for rb in range(nrb):   # row blocks (<=512 rows: one PSUM bank)
                    o0 = rb * rblock
                    n = min(rblock, rc - o0)
                    ps = psum_pool.tile([P, 512], F32, name="ps", tag="ps")
                    for c in range(2 * NSLAB):
                        nc.tensor.matmul(
                            ps[:, :n],
                            w_sb[:, c, j * P : (j + 1) * P],
                            at[:, c, o0 : o0 + n],
                            start=(c == 0),
                            stop=(c == 2 * NSLAB - 1),
                        )
                    nc.vector.tensor_add(
                        ps[:, :n], ps[:, :n], nd[:, j, o0 : o0 + n]
                    )
                    nc.scalar.activation(
                        ot[:, j, o0 : o0 + n],
                        ps[:, :n],
                        mybir.ActivationFunctionType.Relu,
                    )
                # Store per feature-half so the write drains while the other
                # half computes.
                wq.dma_start(
                    out_d[j : j + 1, :, r0 : r0 + rc].rearrange("c p r -> p c r"),
                    ot[:, j : j + 1, :rc],
                )

        nchunk = len(chunks)
        for ci in range(nchunk):
            load_chunk(ci, nc.sync, split=2 if ci == 0 else 1)
        for ci in range(nchunk):
            compute_chunk(ci, rblock=512, wq=nc.gpsimd)

    nc.compile()
    return nc


def get_nc(ntot):
    if ntot not in _nc_cache:
        _nc_cache[ntot] = _build_nc(ntot)
    return _nc_cache[ntot]


def _plan(mol):
    """Balance graphs across cores by valid-row count; build gather indices."""
    mol = np.asarray(mol, dtype=np.int64)
    order = np.argsort(-mol, kind="stable")
    loads = np.zeros(NCORES, dtype=np.int64)
    groups = [[] for _ in range(NCORES)]
    for b in order:
        c = int(np.argmin(loads))
        groups[c].append(int(b))
        loads[c] += mol[b]
    ntot = int(-(-loads.max() // ROW_PAD) * ROW_PAD)
    idx = np.zeros((NCORES, ntot), dtype=np.int64)
    nvalid = np.zeros(NCORES, dtype=np.int64)
    for c in range(NCORES):
        ids = np.concatenate(
            [b * M + np.arange(mol[b]) for b in groups[c]]
        ) if groups[c] else np.zeros(0, dtype=np.int64)
        idx[c, : len(ids)] = ids
        nvalid[c] = len(ids)
    return {"ntot": ntot, "idx": idx, "nvalid": nvalid}


def _packT(flat2d, idx):
    """Gather rows [8, ntot, 256] then lay out as [8, 2, 128, ntot] bf16."""
    g = flat2d[idx]                      # [8, ntot, 256]
    gt = g.transpose(0, 2, 1)            # [8, 256, ntot] (view)
    return np.ascontiguousarray(gt.astype(BF16_NP)).reshape(
        NCORES, 2, P, idx.shape[1]
    )


def plan_and_pack(
    node_features,
    inblock_acts,
    block_outputs,
    mol_slice,
    W_in,
    W_out,
    beta1,
    beta2,
):
    node = np.asarray(node_features, dtype=np.float32).reshape(B * M, F)
    inb = np.asarray(inblock_acts, dtype=np.float32)
    bo = np.asarray(block_outputs, dtype=np.float32)
    mol = np.asarray(mol_slice, dtype=np.int64)
    w_in = np.asarray(W_in, dtype=np.float32)
    w_out = np.asarray(W_out, dtype=np.float32)
    b1 = float(np.asarray(beta1).reshape(-1)[0])
    b2 = float(np.asarray(beta2).reshape(-1)[0])

    plan = _plan(mol)
    idx = plan["idx"]

    wc = (
        np.concatenate([b1 * w_in[0], b1 * w_in[1], b2 * w_out[0]], axis=0)
        .reshape(2 * NSLAB, P, F)
        .astype(BF16_NP)
    )

    nodeT = _packT(node, idx)
    a0T = _packT(inb[0].reshape(B * M, F), idx)
    a1T = _packT(inb[1].reshape(B * M, F), idx)
    boT = _packT(bo[0].reshape(B * M, F), idx)
    ntot = plan["ntot"]

    in_maps = []
    for c in range(NCORES):
        acts = np.empty((2 * NSLAB, P, ntot), dtype=BF16_NP)
        acts[0:2] = a0T[c]
        acts[2:4] = a1T[c]
        acts[4:6] = boT[c]
        in_maps.append(
            {
                "acts": acts,
                "nodet": np.ascontiguousarray(nodeT[c]),
                "wc": wc,
            }
        )
    return plan, in_maps


def unpack(plan, per_core_outs):
    idx, nvalid = plan["idx"], plan["nvalid"]
    ntot = plan["ntot"]
    out_flat = np.zeros((B * M, F), dtype=np.float32)
    for c in range(NCORES):
        o = np.asarray(per_core_outs[c]).reshape(F, ntot)  # [2,128,ntot]->[256,ntot]
        nv = int(nvalid[c])
        out_flat[idx[c, :nv]] = o[:, :nv].T.astype(np.float32)
    return out_flat.reshape(B, M, F)


def kernel(**inputs):
    plan, in_maps = plan_and_pack(**inputs)
    nc = get_nc(plan["ntot"])
    res = bass_utils.run_bass_kernel_spmd(
        nc, in_maps, core_ids=list(range(NCORES))
    )
    return unpack(plan, [res.results[c]["out"] for c in range(NCORES)])
